# revision 62
# baseline (speedup 1.0000x reference)
"""BERT self-attention on 8 Trainium2 NeuronCores (Bass/Tile).

Problem: B=8, S=1024, H=1024, NH=16, HD=64, fp32.
Sharding: pure data-parallel - one batch element per core, weights
replicated. No collectives.

Math notes:
- The attention-mask bias broadcasts over keys ((1-mask)[...,None] is a
  per-(batch,query) constant added to every logit of a softmax row), so
  it cancels exactly in softmax for any finite mask. It is not used.
- Softmax is computed without max-subtraction: logits are ~N(0,1)
  (|max| < ~6), exp is comfortably within fp32 range.
- Projection/scores matmuls run in float32r/bf16; PV runs "natural"
  with the exp output E (bf16) as the stationary operand so the context
  lands in [query, feature] layout and needs no output transpose.
- x and W are converted to bf16 (on the otherwise-idle ACT engine early
  on) before their PE transposes: a bf16 identity + bf16 data stream at
  1.0 cycles/row vs 2.0 for fp32, and the PSUM->SBUF quad copies hit
  the DVE 2x mode. Walrus rejects f32r transposes, so bf16 is the only
  fast-transpose option; the bf16 rounding lands well inside the 2e-2
  tolerance (measured 6.2e-3 on hardware).

Per-core schedule - a skewed software pipeline over head pairs (ot).
Each ot runs 16 paced steps (qb x kt); a step emits the two
64-contraction scores matmuls + the exp for that (qb, kt), then filler
PE work so the PE always holds >= one exp-duration of queued work:
  - PV chunks of the PREVIOUS ot (full-ot skew; E of ot-1 is completely
    exp'd): qb0 chunks during steps 0-7, qb1 chunks during steps 8-15.
    At the tail the skew stretches to 1.5 ots: ot6 keeps only
    PV(ot5,qb0) so its exps finish sooner, and ot7 - whose PE would
    otherwise idle against the exp drain - absorbs PV(ot5,qb1) +
    PV(ot6) + its own qb0 across four group tiles (pvring/tr/acc x2).
  - Weight transposes (steps 0-3) and projection units (one 512-wide
    matmul each) for ot+1, so scores never wait on their own proj.
  - V units (x @ Wv^T for one s-tile, 512 o-cols): block 0 inside ot0,
    block 1 spread over ots 2-4 (ready long before PV(ot4) in ot5).
All loads are single-DMA per 128-row tile (each dma_start costs ~630ns
of serial HWDGE queue time) and ride the SP queue in consumption order
(Wk0/Wq0 first, then x, biases, Wv block 0); Wv block 1 and later
weights issue from inside ot0's steps on the ACT queue. The X phase
pipelines ot0's half-0 projection INSIDE the x-tile DMA cadence: after
each x tile is converted and transposed, its 128-col slice of Kh0/Qh0
accumulates as a sequential per-s-tile PSUM group (128-wide bf16
matmuls cost the same as 512-wide ones), so only one tile's
convert+transpose+projection tail trails the last x DMA. ot0 takes 20
steps to absorb x transposes 4-7, its half-1 projections, V block 0
and proj(ot1). PV normalize is grouped: 4
q-chunks share one reciprocal + one batched store. ot7 absorbs its own
qb0 PV into steps 8-15, and the PV(ot7,qb1) drain accumulates six
chunks kt-major in six PSUM tiles (one pending accumulation group per
tile is the legality limit) so only one wave + an ACT-assisted
normalize trails the final exp.
"""
import numpy as np
from contextlib import ExitStack

import concourse.bass as bass
import concourse.tile as tile
from concourse import bacc, mybir
from concourse.bass_utils import run_bass_kernel_spmd
from concourse.masks import make_identity

B, S, H, NH = 8, 1024, 1024, 16
HD = H // NH          # 64
P = 128
NT = S // P           # 8 s-tiles
HT = H // P           # 8 h-tiles (contraction)
OT = H // P           # 8 o-tiles / head pairs
QBS = 512             # q-block size
VW = HD + 2           # V unit cols: 64 ctx + ones + pad
N_CORES = 8
F32 = mybir.dt.float32
F32R = mybir.dt.float32r
BF16 = mybir.dt.bfloat16
AF = mybir.ActivationFunctionType
ALU = mybir.AluOpType

_CACHE = {}

TUNE = {
    "copy_mode": "dve",   # quad-copy engine: alt | dve | act
    "out_q": "sp",        # out-store DMA queue: act | sp | alt
    "quad_act": False,    # early-region quad copies ride the ACT engine
}


def _emit(tc):
    nc = tc.nc
    # x/W ride as F32R (bit-identical to fp32) end to end so PE transposes
    # can stream a bf16 identity without tripping the fp32-pair constraint
    x = nc.dram_tensor("x", [S, H], F32, kind="ExternalInput").ap()
    wq = nc.dram_tensor("wq", [H, H], F32, kind="ExternalInput").ap()
    wk = nc.dram_tensor("wk", [H, H], F32, kind="ExternalInput").ap()
    wv = nc.dram_tensor("wv", [H, H], F32, kind="ExternalInput").ap()
    bq = nc.dram_tensor("bq", [H], F32, kind="ExternalInput").ap()
    bk = nc.dram_tensor("bk", [H], F32, kind="ExternalInput").ap()
    bv = nc.dram_tensor("bv", [H], F32, kind="ExternalInput").ap()
    out = nc.dram_tensor("out", [S, H], F32, kind="ExternalOutput").ap()
    out_tiled = out.rearrange("(t p) o -> p t o", p=P)

    copy_flip = [0]

    def quad_copy(dst_ap, src_ap):
        mode = TUNE["copy_mode"]
        use_dve = (mode == "dve") or (mode == "alt" and copy_flip[0] % 2 == 0)
        if mode == "act" or not use_dve:
            nc.scalar.copy(dst_ap, src_ap)
        else:
            nc.vector.tensor_copy(dst_ap, src_ap)
        copy_flip[0] += 1

    out_flip = [0]

    def out_dma(dst_ap, src_ap):
        q = TUNE["out_q"]
        use_act = (q == "act") or (q == "alt" and out_flip[0] % 2 == 0)
        (nc.scalar if use_act else nc.sync).dma_start(dst_ap, src_ap)
        out_flip[0] += 1

    with ExitStack() as top:
        consts = top.enter_context(tc.tile_pool(name="consts", bufs=1))
        nat = top.enter_context(tc.tile_pool(name="nat", bufs=10))
        natb = top.enter_context(tc.tile_pool(name="natb", bufs=10))
        big = top.enter_context(tc.tile_pool(name="big", bufs=1))
        wt = top.enter_context(tc.tile_pool(name="wt", bufs=2))
        wtv = top.enter_context(tc.tile_pool(name="wtv", bufs=1))
        qk = top.enter_context(tc.tile_pool(name="qk", bufs=2))
        cp = top.enter_context(tc.tile_pool(name="cp", bufs=6))
        ep = top.enter_context(tc.tile_pool(name="ep", bufs=4))

        # identity first: DVE memset + one gpsimd affine-select, so the
        # first x transposes are not blocked behind slow Pool launches
        ident = consts.tile([P, P], BF16)
        nc.vector.memset(ident[:], 0.0)
        make_identity(nc, ident[:], nomemset=True)

        XT = big.tile([P, HT, S], BF16, tag="XT")    # XT[p, ht, s] = x[s, ht*P+p]
        Vpad = big.tile([P, NT, NH, VW], BF16, tag="Vpad")

        with ExitStack() as phb:
            ps_s = phb.enter_context(tc.tile_pool(name="ps_s", bufs=2, space="PSUM"))
            ps_a = phb.enter_context(tc.tile_pool(name="ps_a", bufs=2, space="PSUM"))
            ps_x = phb.enter_context(tc.tile_pool(name="ps_x", bufs=1, space="PSUM"))

            # ---- DMA staging -------------------------------------------
            # ONE DMA instruction per tile/group: each dma_start costs
            # ~630ns of serial HWDGE queue time regardless of size, so the
            # start is issue-rate-bound if tiles are chunked
            def load_nat(w_ap, ti, q=None):
                wn = nat.tile([P, H], F32, tag="nat")
                src = w_ap.rearrange("(t p) h -> p t h", p=P)
                (q or nc.sync).dma_start(wn[:], src[:, ti, :])
                return wn

            def to_bf(wn, eng="dve"):
                # fp32 -> bf16 staging for the transposes: the bf16 moving
                # identity then streams at 1.0 cycles/row (vs 2.0 for fp32)
                wb = natb.tile([P, H], BF16, tag="natb", name="wnb")
                if eng == "act":
                    nc.scalar.copy(wb[:], wn[:])
                else:
                    nc.vector.tensor_copy(wb[:], wn[:])
                return wb

            # ---- PE transpose quads ------------------------------------
            def quad(wn, q2, dst, dst_cols, tag="tr", eng=None):
                pool = {"s": ps_s, "acc": ps_a, "tr": ps_x}[tag]
                tr = pool.tile([P, 4, P], BF16, tag=tag, name="trq")
                for i in range(4):
                    ht = q2 * 4 + i
                    nc.tensor.transpose(tr[:, i, :], wn[:, ht * P:(ht + 1) * P],
                                        ident[:])
                if eng == "act" and TUNE.get("quad_act", True):
                    nc.scalar.copy(dst[:, q2 * 4:(q2 + 1) * 4, dst_cols], tr[:])
                else:
                    quad_copy(dst[:, q2 * 4:(q2 + 1) * 4, dst_cols], tr[:])

            # ---- projection halves (one matmul per unit() call) --------
            class Half:
                def __init__(self, wT, dst, sb, bias_sb, ot):
                    self.wT, self.dst, self.sb = wT, dst, sb
                    self.bias_sb, self.ot = bias_sb, ot
                    self.acc = None
                    self.ht = 0

                def unit(self):
                    if self.ht == 0:
                        self.acc = ps_a.tile([P, QBS], F32, tag="acc",
                                             name="pacc")
                    ht = self.ht
                    nc.tensor.matmul(
                        self.acc[:], self.wT[:, ht, :],
                        XT[:, ht, self.sb * QBS:(self.sb + 1) * QBS],
                        start=(ht == 0), stop=(ht == HT - 1))
                    self.ht += 1
                    if self.ht == HT:
                        nc.vector.tensor_scalar_add(
                            self.dst[:, self.sb * QBS:(self.sb + 1) * QBS],
                            self.acc[:], self.bias_sb[:, self.ot:self.ot + 1])

            # ---- V units -----------------------------------------------
            wvT_box = [None]
            bv_bc_box = [None]

            def v_unit(st, blk):
                acc = ps_a.tile([P, QBS], F32, tag="acc", name="vacc")
                for ht in range(HT):
                    nc.tensor.matmul(
                        acc[:], XT[:, ht, st * P:(st + 1) * P],
                        wvT_box[0][:, ht, :],
                        start=(ht == 0), stop=(ht == HT - 1))
                nh0 = blk * 8
                nc.vector.tensor_tensor(
                    Vpad[:, st, nh0:nh0 + 8, 0:HD],
                    acc[:].rearrange("p (h d) -> p h d", d=HD),
                    bv_bc_box[0][:, blk * QBS:(blk + 1) * QBS].rearrange(
                        "p (h d) -> p h d", d=HD),
                    ALU.add)

            def v_cols(blk):
                nh0 = blk * 8
                nc.vector.memset(Vpad[:, :, nh0:nh0 + 8, HD:HD + 1], 1.0)
                nc.vector.memset(Vpad[:, :, nh0:nh0 + 8, HD + 1:HD + 2], 0.0)

            # ---- PV units, grouped 4 q-chunks -> 1 normalize + 1 store -
            pvring = ps_x.tile([P, 4, P], F32, tag="pv")

            def finish_group(ot, qb, j, g):
                h = 2 * ot + j
                rc = cp.tile([P, 4], F32, tag="rc", name="rc")
                nc.vector.reciprocal(rc[:], g[:, :, HD])
                ct = cp.tile([P, 4, HD], F32, tag="ct", name="ct")
                for c in range(4):
                    nc.vector.tensor_scalar_mul(
                        ct[:, c, :], g[:, c, 0:HD], rc[:, c:c + 1])
                st0 = qb * 4
                # bv is already inside Vpad: softmax rows sum to 1, so +bv
                # lands as an exact additive bv[d] on the context
                out_dma(out_tiled[:, st0:st0 + 4, h * HD:(h + 1) * HD], ct[:])

            def pv_unit(ot, qb, j, c, E, ring=None, fin="group"):
                h = 2 * ot + j
                g = pvring if ring is None else ring
                pvs = g[:, c, 0:VW]
                for kt in range(NT):
                    nc.tensor.matmul(
                        pvs, E[:, kt, j, c * P:(c + 1) * P],
                        Vpad[:, kt, h, :],
                        start=(kt == 0), stop=(kt == NT - 1))
                if fin == "group":
                    if c == 3:
                        finish_group(ot, qb, j, g)
                    return None
                # fin == "chunk-act": normalize this chunk now, mul on the
                # idle ACT engine; caller stores the grouped ct at c == 3
                return None

            def pv_unit_drain(ot, qb, j, c, E, g, ct, rc):
                h = 2 * ot + j
                pvs = g[:, c, 0:VW]
                for kt in range(NT):
                    nc.tensor.matmul(
                        pvs, E[:, kt, j, c * P:(c + 1) * P],
                        Vpad[:, kt, h, :],
                        start=(kt == 0), stop=(kt == NT - 1))
                nc.vector.reciprocal(rc[:, c:c + 1], g[:, c, HD:HD + 1])
                nc.scalar.activation(ct[:, c, :], g[:, c, 0:HD], AF.Copy,
                                     scale=rc[:, c:c + 1])
                if c == 3:
                    st0 = qb * 4
                    out_dma(out_tiled[:, st0:st0 + 4, h * HD:(h + 1) * HD],
                            ct[:])

            # ---- scores + exp (the pacing pair) ------------------------
            def scores_step(E, qt, kt_, qb, kt):
                ss = ps_s.tile([P, 2, QBS], F32, tag="s", name="ss")
                for j in range(2):
                    pr = slice(j * HD, (j + 1) * HD)
                    nc.tensor.matmul(
                        ss[:, j, :],
                        kt_[pr, kt * P:(kt + 1) * P],
                        qt[pr, qb * QBS:(qb + 1) * QBS],
                        start=True, stop=True)
                nc.scalar.activation(E[:, kt, :, :], ss[:], AF.Exp, scale=0.125)

            # ---- step scheduler ----------------------------------------
            def emit_steps(n_steps, score_slots, pre, post, E01, qt, kt_):
                # never drop scheduled work: run past n_steps if thunks exist
                n_steps = max([n_steps] + [k + 1 for k in pre]
                              + [k + 1 for k in post])
                si = 0
                for s in range(n_steps):
                    for th in pre.get(s, []):
                        th()
                    if s in score_slots:
                        qb, kt = si // NT, si % NT
                        scores_step(E01[qb], qt, kt_, qb, kt)
                        si += 1
                    for th in post.get(s, []):
                        th()

            def add(d, s, th):
                d.setdefault(s, []).append(th)

            # ================= X phase ==================================
            # All start-critical loads ride ONE queue (SP) in exact
            # consumption order - the cost model serializes DMA globally, so
            # any other traffic would stretch the x-tile cadence. wv / w1 /
            # bv issues are deferred into ot0 steps (ACT queue).
            xns = {}
            # Wk0/Wq0 FIRST: their transposes gate ot0's half-0 projection,
            # which then accumulates per s-tile (sequential PSUM groups)
            # inside the x-tile DMA cadence
            wk_n = load_nat(wk, 0)
            wq_n = load_nat(wq, 0)
            xns[0] = load_nat(x, 0)
            xns[1] = load_nat(x, 1)
            xns[2] = load_nat(x, 2)
            xns[3] = load_nat(x, 3)
            bq_sb = consts.tile([P, OT], F32, tag="bq")
            nc.sync.dma_start(bq_sb[:], bq.rearrange("(t p) -> p t", p=P))
            bk_sb = consts.tile([P, OT], F32, tag="bk")
            nc.sync.dma_start(bk_sb[:], bk.rearrange("(t p) -> p t", p=P))
            for st in range(4, NT):
                xns[st] = load_nat(x, st)

            def x_transpose(st, tags):
                for q2 in range(2):
                    quad(xns[st], q2, XT, slice(st * P, (st + 1) * P),
                         tags[q2 % len(tags)])
                del xns[st]

            wk_n = to_bf(wk_n, "act")
            wq_n = to_bf(wq_n, "act")
            wTq = wt.tile([P, HT, P], BF16, tag="wtq")
            wTk = wt.tile([P, HT, P], BF16, tag="wtk")
            quad(wk_n, 0, wTk, slice(0, P), "s")
            quad(wk_n, 1, wTk, slice(0, P), "acc")
            quad(wq_n, 0, wTq, slice(0, P), "tr")
            quad(wq_n, 1, wTq, slice(0, P), "s")

            qt = qk.tile([P, S], F32R, tag="qt")
            kt_ = qk.tile([P, S], F32R, tag="kt")
            accK = ps_a.tile([P, QBS], F32, tag="acc", name="accK")
            accQ = ps_a.tile([P, QBS], F32, tag="acc", name="accQ")

            def kq0_st(st):
                # one s-tile of Kh0+Qh0: 128-wide bf16 matmuls, one
                # sequential accumulation group per (acc, s-tile) region
                sl = slice(st * P, (st + 1) * P)
                for acc, wT in ((accK, wTk), (accQ, wTq)):
                    for ht in range(HT):
                        nc.tensor.matmul(
                            acc[:, sl], wT[:, ht, :], XT[:, ht, sl],
                            start=(ht == 0), stop=(ht == HT - 1))
                # per-s-tile bias-adds: the last tile's chain only carries
                # a 128-col add instead of the full 512-col one
                nc.vector.tensor_scalar_add(
                    kt_[:, sl], accK[:, sl], bk_sb[:, 0:1])
                nc.vector.tensor_scalar_add(
                    qt[:, sl], accQ[:, sl], bq_sb[:, 0:1])

            for st in range(4):
                xns[st] = to_bf(xns[st], "act")
                x_transpose(st, [["s", "tr"], ["s", "tr"],
                                 ["s", "tr"], ["s", "tr"]][st])
                kq0_st(st)

            wv_ns = {}
            for t in range(4):
                wv_ns[t] = load_nat(wv, t)
            w_next = {}
            bv_row = consts.tile([1, H], F32, tag="bv_row")
            nc.sync.dma_start(bv_row[:], bv.unsqueeze(0))

            def dma_wv(t):
                wv_ns[t] = load_nat(wv, t, q=nc.scalar)

            def dma_w1():
                w_next[1] = (load_nat(wq, 1, q=nc.scalar),
                             load_nat(wk, 1, q=nc.scalar))

            def bv_broadcast():
                bv_bc_box[0] = consts.tile([P, H], F32, tag="bv_bc",
                                           name="bv_bc")
                nc.gpsimd.partition_broadcast(bv_bc_box[0][:], bv_row[:])

            # ================= ot0 (22 steps) ===========================
            E0 = ep.tile([P, NT, 2, QBS], BF16, tag="E")
            E1 = ep.tile([P, NT, 2, QBS], BF16, tag="E")
            pre, post = {}, {}

            # deferred DMA issues (ACT queue, spread over early steps)
            add(post, 0, dma_w1)
            for t in range(4, 8):
                add(post, 1 + (t - 4) // 2, (lambda t=t: dma_wv(t)))
            # x tiles 4-7: convert (ACT) then 2 quads/step on the acc slots
            def xconv(st):
                xns[st] = to_bf(xns[st], "act")
            for st in range(4, 8):
                add(pre, st - 4, (lambda st=st: xconv(st)))
                add(post, st - 4, (lambda st=st:
                                   x_transpose(st, ["acc", "acc"])))
            # ot0 half-1 projections: Kh1 steps 4-5, Qh1 steps 6-7 (4/step)
            hK1 = Half(wTk, kt_, 1, bk_sb, 0)
            hQ1 = Half(wTq, qt, 1, bq_sb, 0)
            for s in (4, 4, 4, 4, 5, 5, 5, 5):
                add(post, s, hK1.unit)
            for s in (6, 6, 6, 6, 7, 7, 7, 7):
                add(post, s, hQ1.unit)
            add(post, 4, bv_broadcast)
            # wvT block 0: tr quads 1/step, steps 6-13 (wv lands ~12us)
            def wvt_new():
                wvT_box[0] = wtv.tile([P, HT, 4 * P], BF16, tag="wtv",
                                      name="wvT")
            add(post, 4, wvt_new)
            def wvconv(t, eng="dve"):
                wv_ns[t] = to_bf(wv_ns[t], eng)
            for t in range(4):
                add(pre, 3 + 2 * t, (lambda t=t: wvconv(t)))
            wv_tags0 = ["tr", "tr", "acc", "tr", "acc", "tr", "acc", "tr"]
            for i in range(8):
                add(post, 4 + i, (lambda i=i:
                    quad(wv_ns[i // 2], i % 2, wvT_box[0],
                         slice((i // 2) * P, (i // 2 + 1) * P),
                         wv_tags0[i], "act")))
            # wT(ot1): tr quads steps 14-17
            wTq1 = wt.tile([P, HT, P], BF16, tag="wtq")
            wTk1 = wt.tile([P, HT, P], BF16, tag="wtk")
            for q2 in range(2):
                add(post, 12 + q2, (lambda q2=q2:
                    quad(w_next[1][1], q2, wTk1, slice(0, P), "tr", "act")))
            for q2 in range(2):
                add(post, 14 + q2, (lambda q2=q2:
                    quad(w_next[1][0], q2, wTq1, slice(0, P), "tr", "act")))
            # V block 0: 2 units/step, steps 14-17
            for u in range(8):
                add(post, 12 + u // 2, (lambda u=u: v_unit(u, 0)))
            add(post, 16, (lambda: v_cols(0)))
            # proj(ot1) h0: Kh0 steps 18-19, Qh0 steps 20-21 (4/step)
            qt1 = qk.tile([P, S], F32R, tag="qt")
            kt1 = qk.tile([P, S], F32R, tag="kt")
            hs1 = [Half(wTk1, kt1, 0, bk_sb, 1), Half(wTq1, qt1, 0, bq_sb, 1),
                   Half(wTk1, kt1, 1, bk_sb, 1), Half(wTq1, qt1, 1, bq_sb, 1)]
            for s in (16, 16, 16, 16, 17, 17, 17, 17):
                add(post, s, hs1[0].unit)
            for s in (18, 18, 18, 18, 19, 19, 19, 19):
                add(post, s, hs1[1].unit)
            def wconv(n):
                q_, k_ = w_next[n]
                w_next[n] = (to_bf(q_), to_bf(k_))
            add(pre, 12, (lambda: wconv(1)))
            # wv block-1 transposes start in ot0's tail steps
            add(pre, 14, (lambda: wvconv(4)))
            add(pre, 15, (lambda: wvconv(5)))
            add(post, 16, wvt_new)
            for i in range(4):
                add(post, 16 + i, (lambda i=i:
                    quad(wv_ns[4 + i // 2], i % 2, wvT_box[0],
                         slice((i // 2) * P, (i // 2 + 1) * P), "tr", "act")))
            # weight DMA for ot2
            add(post, 13, (lambda: w_next.__setitem__(
                2, (load_nat(wq, 2), load_nat(wk, 2)))))
            add(pre, 18, (lambda: wconv(2)))

            emit_steps(20, set(range(0, 4)) | set(range(6, 10))
                       | set(range(10, 18)), pre, post, {0: E0, 1: E1},
                       qt, kt_)
            prev_E = (E0, E1)
            prev_E2 = None
            qt, kt_ = qt1, kt1
            own_h1 = (hs1[2], hs1[3])   # ot1 emits its own half-1

            # ================= ots 1..7 =================================
            vsteps = {2: [0, 1, 2], 3: [0, 1, 2], 4: [0, 1]}
            vunits = {2: [0, 1, 2], 3: [3, 4, 5], 4: [6, 7]}
            cur_E = {}

            for ot in range(1, OT):
                E0 = ep.tile([P, NT, 2, QBS], BF16, tag="E")
                E1 = ep.tile([P, NT, 2, QBS], BF16, tag="E")
                cur_E[0], cur_E[1] = E0, E1
                pre, post = {}, {}
                # PV(ot-1): qb0 chunks steps 0-7, qb1 chunks steps 8-15,
                # emitted at the FRONT of each step: the chunks are always
                # ready (E is an ot old), they cover boundary bias-adds, and
                # their normalize lands early in the DVE queue so the ring
                # slice is free when reused. For ots >= 2 the j=1 groups go
                # to the "tr" bank (free after the wT quads) so consecutive
                # groups never share a bank.
                alt_bank = ot >= 2
                gbox = {}

                def pv_prev(qb, j, c, E):
                    if j == 1 and alt_bank:
                        if c == 0:
                            gbox[qb] = ps_x.tile([P, 4, P], F32, tag="tr",
                                                 name="gpv")
                        pv_unit(ot - 1, qb, j, c, E, ring=gbox[qb])
                    else:
                        pv_unit(ot - 1, qb, j, c, E)

                if ot == OT - 2:
                    # 1.5-ot skew at the tail: ot6 keeps only PV(ot5,qb0);
                    # PV(ot5,qb1) moves into ot7's ACT-slack
                    pvl = [(0, j, c) for j in range(2) for c in range(4)]
                elif ot == OT - 1:
                    pvl = []
                else:
                    pvl = [(qb, j, c) for qb in range(2) for j in range(2)
                           for c in range(4)]
                for i, (qb, j, c) in enumerate(pvl):
                    add(pre, i, (lambda qb=qb, j=j, c=c, E=prev_E:
                                 pv_prev(qb, j, c, E[qb])))
                if ot == OT - 1:
                    # steps 0-7: PV(ot5,qb1) on pvring/tr + PV(ot6,qb0) on
                    # the acc tiles (free: ot7 has no projections)
                    g75, g76, g76b = {}, {}, {}

                    def pv75(j, c):
                        if j == 1 and c == 0:
                            g75[1] = ps_x.tile([P, 4, P], F32, tag="tr",
                                               name="g75")
                        pv_unit(OT - 3, 1, j, c, prev_E2[1],
                                ring=g75.get(j))

                    def pv76(j, c):
                        if c == 0:
                            g76[j] = ps_a.tile([P, 4, P], F32, tag="acc",
                                               name="g76")
                        pv_unit(OT - 2, 0, j, c, prev_E[0], ring=g76[j])

                    def pv76b(j, c):
                        if j == 1 and c == 0:
                            g76b[1] = ps_x.tile([P, 4, P], F32, tag="tr",
                                                name="g76b")
                        pv_unit(OT - 2, 1, j, c, prev_E[1],
                                ring=g76b.get(j))

                    pvjc = [(j, c) for j in range(2) for c in range(4)]
                    for i, (j, c) in enumerate(pvjc):
                        add(pre, i, (lambda j=j, c=c: pv75(j, c)))
                        add(post, i, (lambda j=j, c=c: pv76(j, c)))
                        add(pre, 8 + i, (lambda j=j, c=c: pv76b(j, c)))
                if ot == 1:
                    # ot1's own half-1 projections (steps 0-3, 4/step)
                    for s in (0, 0, 0, 0, 1, 1, 1, 1):
                        add(post, s, own_h1[0].unit)
                    for s in (2, 2, 2, 2, 3, 3, 3, 3):
                        add(post, s, own_h1[1].unit)
                if ot < OT - 1:
                    nxt = ot + 1
                    wTq_n = wt.tile([P, HT, P], BF16, tag="wtq", name="wTq_n")
                    wTk_n = wt.tile([P, HT, P], BF16, tag="wtk", name="wTk_n")
                    wqn, wkn = w_next.pop(nxt)
                    for q2 in range(2):
                        add(post, q2, (lambda q2=q2, wn=wkn, d=wTk_n:
                                       quad(wn, q2, d, slice(0, P), "tr")))
                    for q2 in range(2):
                        add(post, 2 + q2, (lambda q2=q2, wn=wqn, d=wTq_n:
                                           quad(wn, q2, d, slice(0, P), "tr")))
                    qtn = qk.tile([P, S], F32R, tag="qt", name="qtn")
                    ktn = qk.tile([P, S], F32R, tag="kt", name="ktn")
                    hs = [Half(wTk_n, ktn, 0, bk_sb, nxt),
                          Half(wTq_n, qtn, 0, bq_sb, nxt),
                          Half(wTk_n, ktn, 1, bk_sb, nxt),
                          Half(wTq_n, qtn, 1, bq_sb, nxt)]
                    p0 = 3 if ot == 1 else 2
                    for s in range(p0, p0 + 4):        # Kh0
                        add(post, s, hs[0].unit)
                        add(post, s, hs[0].unit)
                    for s in range(p0 + 2, p0 + 6):    # Qh0
                        add(post, s, hs[1].unit)
                        add(post, s, hs[1].unit)
                    for s in range(8, 12):             # Kh1
                        add(post, s, hs[2].unit)
                        add(post, s, hs[2].unit)
                    for s in range(11, 15):            # Qh1
                        add(post, s, hs[3].unit)
                        add(post, s, hs[3].unit)
                    if nxt + 1 < OT:
                        add(post, 0, (lambda n=nxt + 1: w_next.__setitem__(
                            n, (load_nat(wq, n), load_nat(wk, n)))))
                        add(pre, 6, (lambda n=nxt + 1: wconv(n)))
                    next_qk = (qtn, ktn)
                else:
                    next_qk = None
                    # ot7: absorb PV(ot7, qb0) into steps 8-15 (E0 is
                    # fully exp'd at step 8); acc slots are free (no proj)
                    g7 = {}

                    def pv7(j, c, g7=g7):
                        if c == 0:
                            g7[j] = ps_a.tile([P, 4, P], F32, tag="acc",
                                              name="g7acc")
                        pv_unit(OT - 1, 0, j, c, cur_E[0], ring=g7[j])

                    for i, (j, c) in enumerate([(j, c) for j in range(2)
                                                for c in range(4)]):
                        add(post, 8 + i, (lambda j=j, c=c: pv7(j, c)))
                if ot == 1:
                    add(pre, 0, (lambda: wvconv(6)))
                    add(pre, 2, (lambda: wvconv(7)))
                    for i in range(4, 8):
                        add(post, 2 * i - 4, (lambda i=i:
                            quad(wv_ns[4 + i // 2], i % 2, wvT_box[0],
                                 slice((i // 2) * P, (i // 2 + 1) * P),
                                 "tr", "act")))
                for s, u in zip(vsteps.get(ot, []), vunits.get(ot, [])):
                    add(pre, s, (lambda u=u: v_unit(u, 1)))
                if ot == 4:
                    add(post, 2, (lambda: v_cols(1)))

                emit_steps(16, set(range(16)), pre, post, {0: E0, 1: E1},
                           qt, kt_)
                prev_E2 = prev_E
                prev_E = (E0, E1)
                if next_qk:
                    qt, kt_ = next_qk

            # ---- drain: PV(ot7, qb1). Six chunks accumulate kt-major in
            # six distinct PSUM tiles (one pending accumulation group per
            # tile is the hardware/interp limit), so after the final exp
            # only one matmul wave + normalize remains; the last two chunks
            # run chunk-major behind them. Muls ride the now-idle ACT.
            E1l = prev_E[1]
            gtiles = [pvring,
                      ps_x.tile([P, 4, P], F32, tag="tr", name="gd_tr"),
                      ps_a.tile([P, 4, P], F32, tag="acc", name="gd_a0"),
                      ps_a.tile([P, 4, P], F32, tag="acc", name="gd_a1"),
                      ps_s.tile([P, 2, QBS], F32, tag="s", name="gd_s0"),
                      ps_s.tile([P, 2, QBS], F32, tag="s", name="gd_s1")]
            ct0 = cp.tile([P, 4, HD], F32, tag="ct", name="ctd0")
            ct1 = cp.tile([P, 4, HD], F32, tag="ct", name="ctd1")
            cts = {0: ct0, 1: ct1}
            chunks = [(j, c) for j in range(2) for c in range(4)]

            def dr_slice(g, c):
                return g[:, c, 0:VW] if g is not pvring else pvring[:, c, 0:VW]

            for kt in range(NT):
                for i, (j, c) in enumerate(chunks[:6]):
                    g = gtiles[i]
                    nc.tensor.matmul(
                        g[:, 0, 0:VW] if i else pvring[:, c, 0:VW],
                        E1l[:, kt, j, c * P:(c + 1) * P],
                        Vpad[:, kt, 2 * (OT - 1) + j, :],
                        start=(kt == 0), stop=(kt == NT - 1))
            for i, (j, c) in enumerate(chunks[6:]):
                g = gtiles[i]
                for kt in range(NT):
                    nc.tensor.matmul(
                        g[:, 1, 0:VW] if i else pvring[:, c, 0:VW],
                        E1l[:, kt, j, c * P:(c + 1) * P],
                        Vpad[:, kt, 2 * (OT - 1) + j, :],
                        start=(kt == 0), stop=(kt == NT - 1))

            def dr_src(i, j, c):
                if i < 6:
                    return gtiles[i][:, 0, 0:VW] if i else pvring[:, c, 0:VW]
                return gtiles[i - 6][:, 1, 0:VW] if i - 6 else pvring[:, c, 0:VW]

            for i, (j, c) in enumerate(chunks):
                src = dr_src(i, j, c)
                rcx = cp.tile([P, 1], F32, tag="rc", name="rcd")
                nc.vector.reciprocal(rcx[:], src[:, HD:HD + 1])
                nc.scalar.activation(cts[j][:, c, :], src[:, 0:HD], AF.Copy,
                                     scale=rcx[:])
            for j in range(2):
                h = 2 * (OT - 1) + j
                out_dma(out_tiled[:, 4:8, h * HD:(h + 1) * HD], cts[j][:])

def build():
    if "nc" in _CACHE:
        return _CACHE["nc"]
    nc = bacc.Bacc("TRN2", target_bir_lowering=False, debug=False,
                   num_devices=N_CORES)
    with tile.TileContext(nc) as tc:
        _emit(tc)
    nc.compile()
    _CACHE["nc"] = nc
    return nc


def make_in_maps(hidden_state, Wq, bq, Wk, bk, Wv, bv):
    hs = np.ascontiguousarray(np.asarray(hidden_state, dtype=np.float32))
    common = {
        "wq": np.ascontiguousarray(np.asarray(Wq, np.float32)),
        "wk": np.ascontiguousarray(np.asarray(Wk, np.float32)),
        "wv": np.ascontiguousarray(np.asarray(Wv, np.float32)),
        "bq": np.ascontiguousarray(np.asarray(bq, np.float32)),
        "bk": np.ascontiguousarray(np.asarray(bk, np.float32)),
        "bv": np.ascontiguousarray(np.asarray(bv, np.float32)),
    }
    return [{"x": hs[i], **common} for i in range(N_CORES)]


def kernel(hidden_state, attention_mask, Wq, bq, Wk, bk, Wv, bv):
    # attention_mask: per-(batch, query) additive constant -> cancels in
    # softmax (see module docstring); unused.
    nc = build()
    in_maps = make_in_maps(hidden_state, Wq, bq, Wk, bk, Wv, bv)
    res = run_bass_kernel_spmd(nc, in_maps, list(range(N_CORES)))
    return np.stack([res.results[i]["out"] for i in range(N_CORES)], axis=0)


# revision 63
# speedup vs baseline: 1.0013x; 1.0013x over previous
"""BERT self-attention on 8 Trainium2 NeuronCores (Bass/Tile).

Problem: B=8, S=1024, H=1024, NH=16, HD=64, fp32.
Sharding: pure data-parallel - one batch element per core, weights
replicated. No collectives.

Math notes:
- The attention-mask bias broadcasts over keys ((1-mask)[...,None] is a
  per-(batch,query) constant added to every logit of a softmax row), so
  it cancels exactly in softmax for any finite mask. It is not used.
- Softmax is computed without max-subtraction: logits are ~N(0,1)
  (|max| < ~6), exp is comfortably within fp32 range.
- Projection/scores matmuls run in float32r/bf16; PV runs "natural"
  with the exp output E (bf16) as the stationary operand so the context
  lands in [query, feature] layout and needs no output transpose.
- x and W are converted to bf16 (on the otherwise-idle ACT engine early
  on) before their PE transposes: a bf16 identity + bf16 data stream at
  1.0 cycles/row vs 2.0 for fp32, and the PSUM->SBUF quad copies hit
  the DVE 2x mode. Walrus rejects f32r transposes, so bf16 is the only
  fast-transpose option; the bf16 rounding lands well inside the 2e-2
  tolerance (measured 6.2e-3 on hardware).

Per-core schedule - a skewed software pipeline over head pairs (ot).
Each ot runs 16 paced steps (qb x kt); a step emits the two
64-contraction scores matmuls + the exp for that (qb, kt), then filler
PE work so the PE always holds >= one exp-duration of queued work:
  - PV chunks of the PREVIOUS ot (full-ot skew; E of ot-1 is completely
    exp'd): qb0 chunks during steps 0-7, qb1 chunks during steps 8-15.
    At the tail the skew stretches to 1.5 ots: ot6 keeps only
    PV(ot5,qb0) so its exps finish sooner, and ot7 - whose PE would
    otherwise idle against the exp drain - absorbs PV(ot5,qb1) +
    PV(ot6) + its own qb0 across four group tiles (pvring/tr/acc x2).
  - Weight transposes (steps 0-3) and projection units (one 512-wide
    matmul each) for ot+1, so scores never wait on their own proj.
  - V units (x @ Wv^T for one s-tile, 512 o-cols): block 0 inside ot0,
    block 1 spread over ots 2-4 (ready long before PV(ot4) in ot5).
All loads are single-DMA per 128-row tile (each dma_start costs ~630ns
of serial HWDGE queue time) and ride the SP queue in consumption order
(Wk0/Wq0 first, then x, biases, Wv block 0); Wv block 1 and later
weights issue from inside ot0's steps on the ACT queue. The X phase
pipelines ot0's half-0 projection INSIDE the x-tile DMA cadence: after
each x tile is converted and transposed, its 128-col slice of Kh0/Qh0
accumulates as a sequential per-s-tile PSUM group (128-wide bf16
matmuls cost the same as 512-wide ones), so only one tile's
convert+transpose+projection tail trails the last x DMA. ot0 takes 20
steps to absorb x transposes 4-7, its half-1 projections, V block 0
and proj(ot1). PV normalize is grouped: 4
q-chunks share one reciprocal + one batched store. ot7 absorbs its own
qb0 PV into steps 8-15, and the PV(ot7,qb1) drain accumulates six
chunks kt-major in six PSUM tiles (one pending accumulation group per
tile is the legality limit) so only one wave + an ACT-assisted
normalize trails the final exp.
"""
import numpy as np
from contextlib import ExitStack

import concourse.bass as bass
import concourse.tile as tile
from concourse import bacc, mybir
from concourse.bass_utils import run_bass_kernel_spmd
from concourse.masks import make_identity

B, S, H, NH = 8, 1024, 1024, 16
HD = H // NH          # 64
P = 128
NT = S // P           # 8 s-tiles
HT = H // P           # 8 h-tiles (contraction)
OT = H // P           # 8 o-tiles / head pairs
QBS = 512             # q-block size
VW = HD + 2           # V unit cols: 64 ctx + ones + pad
N_CORES = 8
F32 = mybir.dt.float32
F32R = mybir.dt.float32r
BF16 = mybir.dt.bfloat16
AF = mybir.ActivationFunctionType
ALU = mybir.AluOpType

_CACHE = {}

TUNE = {
    "copy_mode": "dve",   # quad-copy engine: alt | dve | act
    "out_q": "sp",        # out-store DMA queue: act | sp | alt
    "quad_act": False,    # early-region quad copies ride the ACT engine
}


def _emit(tc):
    nc = tc.nc
    # x/W ride as F32R (bit-identical to fp32) end to end so PE transposes
    # can stream a bf16 identity without tripping the fp32-pair constraint
    x = nc.dram_tensor("x", [S, H], F32, kind="ExternalInput").ap()
    wq = nc.dram_tensor("wq", [H, H], F32, kind="ExternalInput").ap()
    wk = nc.dram_tensor("wk", [H, H], F32, kind="ExternalInput").ap()
    wv = nc.dram_tensor("wv", [H, H], F32, kind="ExternalInput").ap()
    bq = nc.dram_tensor("bq", [H], F32, kind="ExternalInput").ap()
    bk = nc.dram_tensor("bk", [H], F32, kind="ExternalInput").ap()
    bv = nc.dram_tensor("bv", [H], F32, kind="ExternalInput").ap()
    out = nc.dram_tensor("out", [S, H], F32, kind="ExternalOutput").ap()
    out_tiled = out.rearrange("(t p) o -> p t o", p=P)

    copy_flip = [0]

    def quad_copy(dst_ap, src_ap):
        mode = TUNE["copy_mode"]
        use_dve = (mode == "dve") or (mode == "alt" and copy_flip[0] % 2 == 0)
        if mode == "act" or not use_dve:
            nc.scalar.copy(dst_ap, src_ap)
        else:
            nc.vector.tensor_copy(dst_ap, src_ap)
        copy_flip[0] += 1

    out_flip = [0]

    def out_dma(dst_ap, src_ap):
        q = TUNE["out_q"]
        use_act = (q == "act") or (q == "alt" and out_flip[0] % 2 == 0)
        (nc.scalar if use_act else nc.sync).dma_start(dst_ap, src_ap)
        out_flip[0] += 1

    with ExitStack() as top:
        consts = top.enter_context(tc.tile_pool(name="consts", bufs=1))
        nat = top.enter_context(tc.tile_pool(name="nat", bufs=10))
        natb = top.enter_context(tc.tile_pool(name="natb", bufs=10))
        big = top.enter_context(tc.tile_pool(name="big", bufs=1))
        wt = top.enter_context(tc.tile_pool(name="wt", bufs=2))
        wtv = top.enter_context(tc.tile_pool(name="wtv", bufs=1))
        qk = top.enter_context(tc.tile_pool(name="qk", bufs=2))
        cp = top.enter_context(tc.tile_pool(name="cp", bufs=6))
        ep = top.enter_context(tc.tile_pool(name="ep", bufs=4))

        # identity first: DVE memset + one gpsimd affine-select, so the
        # first x transposes are not blocked behind slow Pool launches
        ident = consts.tile([P, P], BF16)
        nc.vector.memset(ident[:], 0.0)
        make_identity(nc, ident[:], nomemset=True)

        XT = big.tile([P, HT, S], BF16, tag="XT")    # XT[p, ht, s] = x[s, ht*P+p]
        Vpad = big.tile([P, NT, NH, VW], BF16, tag="Vpad")

        with ExitStack() as phb:
            ps_s = phb.enter_context(tc.tile_pool(name="ps_s", bufs=2, space="PSUM"))
            ps_a = phb.enter_context(tc.tile_pool(name="ps_a", bufs=2, space="PSUM"))
            ps_x = phb.enter_context(tc.tile_pool(name="ps_x", bufs=1, space="PSUM"))

            # ---- DMA staging -------------------------------------------
            # ONE DMA instruction per tile/group: each dma_start costs
            # ~630ns of serial HWDGE queue time regardless of size, so the
            # start is issue-rate-bound if tiles are chunked
            def load_nat(w_ap, ti, q=None):
                wn = nat.tile([P, H], F32, tag="nat")
                src = w_ap.rearrange("(t p) h -> p t h", p=P)
                (q or nc.sync).dma_start(wn[:], src[:, ti, :])
                return wn

            def to_bf(wn, eng="dve"):
                # fp32 -> bf16 staging for the transposes: the bf16 moving
                # identity then streams at 1.0 cycles/row (vs 2.0 for fp32)
                wb = natb.tile([P, H], BF16, tag="natb", name="wnb")
                if eng == "act":
                    nc.scalar.copy(wb[:], wn[:])
                else:
                    nc.vector.tensor_copy(wb[:], wn[:])
                return wb

            # ---- PE transpose quads ------------------------------------
            def quad(wn, q2, dst, dst_cols, tag="tr", eng=None):
                pool = {"s": ps_s, "acc": ps_a, "tr": ps_x}[tag]
                tr = pool.tile([P, 4, P], BF16, tag=tag, name="trq")
                for i in range(4):
                    ht = q2 * 4 + i
                    nc.tensor.transpose(tr[:, i, :], wn[:, ht * P:(ht + 1) * P],
                                        ident[:])
                if eng == "act" and TUNE.get("quad_act", True):
                    nc.scalar.copy(dst[:, q2 * 4:(q2 + 1) * 4, dst_cols], tr[:])
                else:
                    quad_copy(dst[:, q2 * 4:(q2 + 1) * 4, dst_cols], tr[:])

            # ---- projection halves (one matmul per unit() call) --------
            class Half:
                def __init__(self, wT, dst, sb, bias_sb, ot):
                    self.wT, self.dst, self.sb = wT, dst, sb
                    self.bias_sb, self.ot = bias_sb, ot
                    self.acc = None
                    self.ht = 0

                def unit(self):
                    if self.ht == 0:
                        self.acc = ps_a.tile([P, QBS], F32, tag="acc",
                                             name="pacc")
                    ht = self.ht
                    nc.tensor.matmul(
                        self.acc[:], self.wT[:, ht, :],
                        XT[:, ht, self.sb * QBS:(self.sb + 1) * QBS],
                        start=(ht == 0), stop=(ht == HT - 1))
                    self.ht += 1
                    if self.ht == HT:
                        nc.vector.tensor_scalar_add(
                            self.dst[:, self.sb * QBS:(self.sb + 1) * QBS],
                            self.acc[:], self.bias_sb[:, self.ot:self.ot + 1])

            # ---- V units -----------------------------------------------
            wvT_box = [None]
            bv_bc_box = [None]

            def v_unit(st, blk):
                acc = ps_a.tile([P, QBS], F32, tag="acc", name="vacc")
                for ht in range(HT):
                    nc.tensor.matmul(
                        acc[:], XT[:, ht, st * P:(st + 1) * P],
                        wvT_box[0][:, ht, :],
                        start=(ht == 0), stop=(ht == HT - 1))
                nh0 = blk * 8
                nc.vector.tensor_tensor(
                    Vpad[:, st, nh0:nh0 + 8, 0:HD],
                    acc[:].rearrange("p (h d) -> p h d", d=HD),
                    bv_bc_box[0][:, blk * QBS:(blk + 1) * QBS].rearrange(
                        "p (h d) -> p h d", d=HD),
                    ALU.add)

            def v_cols(blk):
                nh0 = blk * 8
                nc.vector.memset(Vpad[:, :, nh0:nh0 + 8, HD:HD + 1], 1.0)
                nc.vector.memset(Vpad[:, :, nh0:nh0 + 8, HD + 1:HD + 2], 0.0)

            # ---- PV units, grouped 4 q-chunks -> 1 normalize + 1 store -
            pvring = ps_x.tile([P, 4, P], F32, tag="pv")

            def finish_group(ot, qb, j, g):
                h = 2 * ot + j
                rc = cp.tile([P, 4], F32, tag="rc", name="rc")
                nc.vector.reciprocal(rc[:], g[:, :, HD])
                ct = cp.tile([P, 4, HD], F32, tag="ct", name="ct")
                for c in range(4):
                    nc.vector.tensor_scalar_mul(
                        ct[:, c, :], g[:, c, 0:HD], rc[:, c:c + 1])
                st0 = qb * 4
                # bv is already inside Vpad: softmax rows sum to 1, so +bv
                # lands as an exact additive bv[d] on the context
                out_dma(out_tiled[:, st0:st0 + 4, h * HD:(h + 1) * HD], ct[:])

            def pv_unit(ot, qb, j, c, E, ring=None, fin="group"):
                h = 2 * ot + j
                g = pvring if ring is None else ring
                pvs = g[:, c, 0:VW]
                for kt in range(NT):
                    nc.tensor.matmul(
                        pvs, E[:, kt, j, c * P:(c + 1) * P],
                        Vpad[:, kt, h, :],
                        start=(kt == 0), stop=(kt == NT - 1))
                if fin == "group":
                    if c == 3:
                        finish_group(ot, qb, j, g)
                    return None
                # fin == "chunk-act": normalize this chunk now, mul on the
                # idle ACT engine; caller stores the grouped ct at c == 3
                return None

            def pv_unit_drain(ot, qb, j, c, E, g, ct, rc):
                h = 2 * ot + j
                pvs = g[:, c, 0:VW]
                for kt in range(NT):
                    nc.tensor.matmul(
                        pvs, E[:, kt, j, c * P:(c + 1) * P],
                        Vpad[:, kt, h, :],
                        start=(kt == 0), stop=(kt == NT - 1))
                nc.vector.reciprocal(rc[:, c:c + 1], g[:, c, HD:HD + 1])
                nc.scalar.activation(ct[:, c, :], g[:, c, 0:HD], AF.Copy,
                                     scale=rc[:, c:c + 1])
                if c == 3:
                    st0 = qb * 4
                    out_dma(out_tiled[:, st0:st0 + 4, h * HD:(h + 1) * HD],
                            ct[:])

            # ---- scores + exp (the pacing pair) ------------------------
            def scores_step(E, qt, kt_, qb, kt):
                ss = ps_s.tile([P, 2, QBS], F32, tag="s", name="ss")
                for j in range(2):
                    pr = slice(j * HD, (j + 1) * HD)
                    nc.tensor.matmul(
                        ss[:, j, :],
                        kt_[pr, kt * P:(kt + 1) * P],
                        qt[pr, qb * QBS:(qb + 1) * QBS],
                        start=True, stop=True)
                nc.scalar.activation(E[:, kt, :, :], ss[:], AF.Exp, scale=0.125)

            # ---- step scheduler ----------------------------------------
            def emit_steps(n_steps, score_slots, pre, post, E01, qt, kt_):
                # never drop scheduled work: run past n_steps if thunks exist
                n_steps = max([n_steps] + [k + 1 for k in pre]
                              + [k + 1 for k in post])
                si = 0
                for s in range(n_steps):
                    for th in pre.get(s, []):
                        th()
                    if s in score_slots:
                        qb, kt = si // NT, si % NT
                        scores_step(E01[qb], qt, kt_, qb, kt)
                        si += 1
                    for th in post.get(s, []):
                        th()

            def add(d, s, th):
                d.setdefault(s, []).append(th)

            # ================= X phase ==================================
            # All start-critical loads ride ONE queue (SP) in exact
            # consumption order - the cost model serializes DMA globally, so
            # any other traffic would stretch the x-tile cadence. wv / w1 /
            # bv issues are deferred into ot0 steps (ACT queue).
            xns = {}
            # Wk0/Wq0 FIRST: their transposes gate ot0's half-0 projection,
            # which then accumulates per s-tile (sequential PSUM groups)
            # inside the x-tile DMA cadence
            wk_n = load_nat(wk, 0)
            wq_n = load_nat(wq, 0)
            xns[0] = load_nat(x, 0)
            xns[1] = load_nat(x, 1)
            xns[2] = load_nat(x, 2)
            xns[3] = load_nat(x, 3)
            bq_sb = consts.tile([P, OT], F32, tag="bq")
            nc.sync.dma_start(bq_sb[:], bq.rearrange("(t p) -> p t", p=P))
            bk_sb = consts.tile([P, OT], F32, tag="bk")
            nc.sync.dma_start(bk_sb[:], bk.rearrange("(t p) -> p t", p=P))
            for st in range(4, NT):
                xns[st] = load_nat(x, st)

            def x_transpose(st, tags):
                for q2 in range(2):
                    quad(xns[st], q2, XT, slice(st * P, (st + 1) * P),
                         tags[q2 % len(tags)])
                del xns[st]

            wk_n = to_bf(wk_n, "act")
            wq_n = to_bf(wq_n, "act")
            wTq = wt.tile([P, HT, P], BF16, tag="wtq")
            wTk = wt.tile([P, HT, P], BF16, tag="wtk")
            quad(wk_n, 0, wTk, slice(0, P), "s")
            quad(wk_n, 1, wTk, slice(0, P), "acc")
            quad(wq_n, 0, wTq, slice(0, P), "tr")
            quad(wq_n, 1, wTq, slice(0, P), "s")

            qt = qk.tile([P, S], F32R, tag="qt")
            kt_ = qk.tile([P, S], F32R, tag="kt")
            accK = ps_a.tile([P, QBS], F32, tag="acc", name="accK")
            accQ = ps_a.tile([P, QBS], F32, tag="acc", name="accQ")

            def kq0_st(st):
                # one s-tile of Kh0+Qh0: 128-wide bf16 matmuls, one
                # sequential accumulation group per (acc, s-tile) region
                sl = slice(st * P, (st + 1) * P)
                for acc, wT in ((accK, wTk), (accQ, wTq)):
                    for ht in range(HT):
                        nc.tensor.matmul(
                            acc[:, sl], wT[:, ht, :], XT[:, ht, sl],
                            start=(ht == 0), stop=(ht == HT - 1))
                # bias-adds: per-tile for st0/st1, but st2+st3 defer and
                # merge after the last matmul - an intervening PSUM read
                # between consecutive accumulation groups stalls the next
                # group's start behind the DVE queue
                if st < 2:
                    nc.vector.tensor_scalar_add(
                        kt_[:, sl], accK[:, sl], bk_sb[:, 0:1])
                    nc.vector.tensor_scalar_add(
                        qt[:, sl], accQ[:, sl], bq_sb[:, 0:1])
                elif st == 3:
                    tl = slice(2 * P, 4 * P)
                    nc.vector.tensor_scalar_add(
                        kt_[:, tl], accK[:, tl], bk_sb[:, 0:1])
                    nc.vector.tensor_scalar_add(
                        qt[:, tl], accQ[:, tl], bq_sb[:, 0:1])

            for st in range(4):
                xns[st] = to_bf(xns[st], "act")
                x_transpose(st, [["s", "tr"], ["s", "tr"],
                                 ["s", "tr"], ["s", "tr"]][st])
                kq0_st(st)

            wv_ns = {}
            for t in range(4):
                wv_ns[t] = load_nat(wv, t)
            w_next = {}
            bv_row = consts.tile([1, H], F32, tag="bv_row")
            nc.sync.dma_start(bv_row[:], bv.unsqueeze(0))

            def dma_wv(t):
                wv_ns[t] = load_nat(wv, t, q=nc.scalar)

            def dma_w1():
                w_next[1] = (load_nat(wq, 1, q=nc.scalar),
                             load_nat(wk, 1, q=nc.scalar))

            def bv_broadcast():
                bv_bc_box[0] = consts.tile([P, H], F32, tag="bv_bc",
                                           name="bv_bc")
                nc.gpsimd.partition_broadcast(bv_bc_box[0][:], bv_row[:])

            # ================= ot0 (22 steps) ===========================
            E0 = ep.tile([P, NT, 2, QBS], BF16, tag="E")
            E1 = ep.tile([P, NT, 2, QBS], BF16, tag="E")
            pre, post = {}, {}

            # deferred DMA issues (ACT queue, spread over early steps)
            add(post, 0, dma_w1)
            for t in range(4, 8):
                add(post, 1 + (t - 4) // 2, (lambda t=t: dma_wv(t)))
            # x tiles 4-7: convert (ACT) then 2 quads/step on the acc slots
            def xconv(st):
                xns[st] = to_bf(xns[st], "act")
            for st in range(4, 8):
                add(pre, st - 4, (lambda st=st: xconv(st)))
                add(post, st - 4, (lambda st=st:
                                   x_transpose(st, ["acc", "acc"])))
            # ot0 half-1 projections: Kh1 steps 4-5, Qh1 steps 6-7 (4/step)
            hK1 = Half(wTk, kt_, 1, bk_sb, 0)
            hQ1 = Half(wTq, qt, 1, bq_sb, 0)
            for s in (4, 4, 4, 4, 5, 5, 5, 5):
                add(post, s, hK1.unit)
            for s in (6, 6, 6, 6, 7, 7, 7, 7):
                add(post, s, hQ1.unit)
            add(post, 4, bv_broadcast)
            # wvT block 0: tr quads 1/step, steps 6-13 (wv lands ~12us)
            def wvt_new():
                wvT_box[0] = wtv.tile([P, HT, 4 * P], BF16, tag="wtv",
                                      name="wvT")
            add(post, 4, wvt_new)
            def wvconv(t, eng="dve"):
                wv_ns[t] = to_bf(wv_ns[t], eng)
            for t in range(4):
                add(pre, 3 + 2 * t, (lambda t=t: wvconv(t)))
            wv_tags0 = ["tr", "tr", "acc", "tr", "acc", "tr", "acc", "tr"]
            for i in range(8):
                add(post, 4 + i, (lambda i=i:
                    quad(wv_ns[i // 2], i % 2, wvT_box[0],
                         slice((i // 2) * P, (i // 2 + 1) * P),
                         wv_tags0[i], "act")))
            # wT(ot1): tr quads steps 14-17
            wTq1 = wt.tile([P, HT, P], BF16, tag="wtq")
            wTk1 = wt.tile([P, HT, P], BF16, tag="wtk")
            for q2 in range(2):
                add(post, 12 + q2, (lambda q2=q2:
                    quad(w_next[1][1], q2, wTk1, slice(0, P), "tr", "act")))
            for q2 in range(2):
                add(post, 14 + q2, (lambda q2=q2:
                    quad(w_next[1][0], q2, wTq1, slice(0, P), "tr", "act")))
            # V block 0: 2 units/step, steps 14-17
            for u in range(8):
                add(post, 12 + u // 2, (lambda u=u: v_unit(u, 0)))
            add(post, 16, (lambda: v_cols(0)))
            # proj(ot1) h0: Kh0 steps 18-19, Qh0 steps 20-21 (4/step)
            qt1 = qk.tile([P, S], F32R, tag="qt")
            kt1 = qk.tile([P, S], F32R, tag="kt")
            hs1 = [Half(wTk1, kt1, 0, bk_sb, 1), Half(wTq1, qt1, 0, bq_sb, 1),
                   Half(wTk1, kt1, 1, bk_sb, 1), Half(wTq1, qt1, 1, bq_sb, 1)]
            for s in (16, 16, 16, 16, 17, 17, 17, 17):
                add(post, s, hs1[0].unit)
            for s in (18, 18, 18, 18, 19, 19, 19, 19):
                add(post, s, hs1[1].unit)
            def wconv(n):
                q_, k_ = w_next[n]
                w_next[n] = (to_bf(q_), to_bf(k_))
            add(pre, 12, (lambda: wconv(1)))
            # wv block-1 transposes start in ot0's tail steps
            add(pre, 14, (lambda: wvconv(4)))
            add(pre, 15, (lambda: wvconv(5)))
            add(post, 16, wvt_new)
            for i in range(4):
                add(post, 16 + i, (lambda i=i:
                    quad(wv_ns[4 + i // 2], i % 2, wvT_box[0],
                         slice((i // 2) * P, (i // 2 + 1) * P), "tr", "act")))
            # weight DMA for ot2
            add(post, 13, (lambda: w_next.__setitem__(
                2, (load_nat(wq, 2), load_nat(wk, 2)))))
            add(pre, 18, (lambda: wconv(2)))

            emit_steps(20, set(range(0, 4)) | set(range(6, 10))
                       | set(range(10, 18)), pre, post, {0: E0, 1: E1},
                       qt, kt_)
            prev_E = (E0, E1)
            prev_E2 = None
            qt, kt_ = qt1, kt1
            own_h1 = (hs1[2], hs1[3])   # ot1 emits its own half-1

            # ================= ots 1..7 =================================
            vsteps = {2: [0, 1, 2], 3: [0, 1, 2], 4: [0, 1]}
            vunits = {2: [0, 1, 2], 3: [3, 4, 5], 4: [6, 7]}
            cur_E = {}

            for ot in range(1, OT):
                E0 = ep.tile([P, NT, 2, QBS], BF16, tag="E")
                E1 = ep.tile([P, NT, 2, QBS], BF16, tag="E")
                cur_E[0], cur_E[1] = E0, E1
                pre, post = {}, {}
                # PV(ot-1): qb0 chunks steps 0-7, qb1 chunks steps 8-15,
                # emitted at the FRONT of each step: the chunks are always
                # ready (E is an ot old), they cover boundary bias-adds, and
                # their normalize lands early in the DVE queue so the ring
                # slice is free when reused. For ots >= 2 the j=1 groups go
                # to the "tr" bank (free after the wT quads) so consecutive
                # groups never share a bank.
                alt_bank = ot >= 2
                gbox = {}

                def pv_prev(qb, j, c, E):
                    if j == 1 and alt_bank:
                        if c == 0:
                            gbox[qb] = ps_x.tile([P, 4, P], F32, tag="tr",
                                                 name="gpv")
                        pv_unit(ot - 1, qb, j, c, E, ring=gbox[qb])
                    else:
                        pv_unit(ot - 1, qb, j, c, E)

                if ot == OT - 2:
                    # 1.5-ot skew at the tail: ot6 keeps only PV(ot5,qb0);
                    # PV(ot5,qb1) moves into ot7's ACT-slack
                    pvl = [(0, j, c) for j in range(2) for c in range(4)]
                elif ot == OT - 1:
                    pvl = []
                else:
                    pvl = [(qb, j, c) for qb in range(2) for j in range(2)
                           for c in range(4)]
                for i, (qb, j, c) in enumerate(pvl):
                    add(pre, i, (lambda qb=qb, j=j, c=c, E=prev_E:
                                 pv_prev(qb, j, c, E[qb])))
                if ot == OT - 1:
                    # steps 0-7: PV(ot5,qb1) on pvring/tr + PV(ot6,qb0) on
                    # the acc tiles (free: ot7 has no projections)
                    g75, g76, g76b = {}, {}, {}

                    def pv75(j, c):
                        if j == 1 and c == 0:
                            g75[1] = ps_x.tile([P, 4, P], F32, tag="tr",
                                               name="g75")
                        pv_unit(OT - 3, 1, j, c, prev_E2[1],
                                ring=g75.get(j))

                    def pv76(j, c):
                        if c == 0:
                            g76[j] = ps_a.tile([P, 4, P], F32, tag="acc",
                                               name="g76")
                        pv_unit(OT - 2, 0, j, c, prev_E[0], ring=g76[j])

                    def pv76b(j, c):
                        if j == 1 and c == 0:
                            g76b[1] = ps_x.tile([P, 4, P], F32, tag="tr",
                                                name="g76b")
                        pv_unit(OT - 2, 1, j, c, prev_E[1],
                                ring=g76b.get(j))

                    pvjc = [(j, c) for j in range(2) for c in range(4)]
                    for i, (j, c) in enumerate(pvjc):
                        add(pre, i, (lambda j=j, c=c: pv75(j, c)))
                        add(post, i, (lambda j=j, c=c: pv76(j, c)))
                        add(pre, 8 + i, (lambda j=j, c=c: pv76b(j, c)))
                if ot == 1:
                    # ot1's own half-1 projections (steps 0-3, 4/step)
                    for s in (0, 0, 0, 0, 1, 1, 1, 1):
                        add(post, s, own_h1[0].unit)
                    for s in (2, 2, 2, 2, 3, 3, 3, 3):
                        add(post, s, own_h1[1].unit)
                if ot < OT - 1:
                    nxt = ot + 1
                    wTq_n = wt.tile([P, HT, P], BF16, tag="wtq", name="wTq_n")
                    wTk_n = wt.tile([P, HT, P], BF16, tag="wtk", name="wTk_n")
                    wqn, wkn = w_next.pop(nxt)
                    for q2 in range(2):
                        add(post, q2, (lambda q2=q2, wn=wkn, d=wTk_n:
                                       quad(wn, q2, d, slice(0, P), "tr")))
                    for q2 in range(2):
                        add(post, 2 + q2, (lambda q2=q2, wn=wqn, d=wTq_n:
                                           quad(wn, q2, d, slice(0, P), "tr")))
                    qtn = qk.tile([P, S], F32R, tag="qt", name="qtn")
                    ktn = qk.tile([P, S], F32R, tag="kt", name="ktn")
                    hs = [Half(wTk_n, ktn, 0, bk_sb, nxt),
                          Half(wTq_n, qtn, 0, bq_sb, nxt),
                          Half(wTk_n, ktn, 1, bk_sb, nxt),
                          Half(wTq_n, qtn, 1, bq_sb, nxt)]
                    p0 = 3 if ot == 1 else 2
                    for s in range(p0, p0 + 4):        # Kh0
                        add(post, s, hs[0].unit)
                        add(post, s, hs[0].unit)
                    for s in range(p0 + 2, p0 + 6):    # Qh0
                        add(post, s, hs[1].unit)
                        add(post, s, hs[1].unit)
                    for s in range(8, 12):             # Kh1
                        add(post, s, hs[2].unit)
                        add(post, s, hs[2].unit)
                    for s in range(11, 15):            # Qh1
                        add(post, s, hs[3].unit)
                        add(post, s, hs[3].unit)
                    if nxt + 1 < OT:
                        add(post, 0, (lambda n=nxt + 1: w_next.__setitem__(
                            n, (load_nat(wq, n), load_nat(wk, n)))))
                        add(pre, 6, (lambda n=nxt + 1: wconv(n)))
                    next_qk = (qtn, ktn)
                else:
                    next_qk = None
                    # ot7: absorb PV(ot7, qb0) into steps 8-15 (E0 is
                    # fully exp'd at step 8); acc slots are free (no proj)
                    g7 = {}

                    def pv7(j, c, g7=g7):
                        if c == 0:
                            g7[j] = ps_a.tile([P, 4, P], F32, tag="acc",
                                              name="g7acc")
                        pv_unit(OT - 1, 0, j, c, cur_E[0], ring=g7[j])

                    for i, (j, c) in enumerate([(j, c) for j in range(2)
                                                for c in range(4)]):
                        add(post, 8 + i, (lambda j=j, c=c: pv7(j, c)))
                if ot == 1:
                    add(pre, 0, (lambda: wvconv(6)))
                    add(pre, 2, (lambda: wvconv(7)))
                    for i in range(4, 8):
                        add(post, 2 * i - 4, (lambda i=i:
                            quad(wv_ns[4 + i // 2], i % 2, wvT_box[0],
                                 slice((i // 2) * P, (i // 2 + 1) * P),
                                 "tr", "act")))
                for s, u in zip(vsteps.get(ot, []), vunits.get(ot, [])):
                    add(pre, s, (lambda u=u: v_unit(u, 1)))
                if ot == 4:
                    add(post, 2, (lambda: v_cols(1)))

                emit_steps(16, set(range(16)), pre, post, {0: E0, 1: E1},
                           qt, kt_)
                prev_E2 = prev_E
                prev_E = (E0, E1)
                if next_qk:
                    qt, kt_ = next_qk

            # ---- drain: PV(ot7, qb1). Six chunks accumulate kt-major in
            # six distinct PSUM tiles (one pending accumulation group per
            # tile is the hardware/interp limit), so after the final exp
            # only one matmul wave + normalize remains; the last two chunks
            # run chunk-major behind them. Muls ride the now-idle ACT.
            E1l = prev_E[1]
            gtiles = [pvring,
                      ps_x.tile([P, 4, P], F32, tag="tr", name="gd_tr"),
                      ps_a.tile([P, 4, P], F32, tag="acc", name="gd_a0"),
                      ps_a.tile([P, 4, P], F32, tag="acc", name="gd_a1"),
                      ps_s.tile([P, 2, QBS], F32, tag="s", name="gd_s0"),
                      ps_s.tile([P, 2, QBS], F32, tag="s", name="gd_s1")]
            ct0 = cp.tile([P, 4, HD], F32, tag="ct", name="ctd0")
            ct1 = cp.tile([P, 4, HD], F32, tag="ct", name="ctd1")
            cts = {0: ct0, 1: ct1}
            chunks = [(j, c) for j in range(2) for c in range(4)]

            def dr_slice(g, c):
                return g[:, c, 0:VW] if g is not pvring else pvring[:, c, 0:VW]

            for kt in range(NT):
                for i, (j, c) in enumerate(chunks[:6]):
                    g = gtiles[i]
                    nc.tensor.matmul(
                        g[:, 0, 0:VW] if i else pvring[:, c, 0:VW],
                        E1l[:, kt, j, c * P:(c + 1) * P],
                        Vpad[:, kt, 2 * (OT - 1) + j, :],
                        start=(kt == 0), stop=(kt == NT - 1))
            for i, (j, c) in enumerate(chunks[6:]):
                g = gtiles[i]
                for kt in range(NT):
                    nc.tensor.matmul(
                        g[:, 1, 0:VW] if i else pvring[:, c, 0:VW],
                        E1l[:, kt, j, c * P:(c + 1) * P],
                        Vpad[:, kt, 2 * (OT - 1) + j, :],
                        start=(kt == 0), stop=(kt == NT - 1))

            def dr_src(i, j, c):
                if i < 6:
                    return gtiles[i][:, 0, 0:VW] if i else pvring[:, c, 0:VW]
                return gtiles[i - 6][:, 1, 0:VW] if i - 6 else pvring[:, c, 0:VW]

            for i, (j, c) in enumerate(chunks):
                src = dr_src(i, j, c)
                rcx = cp.tile([P, 1], F32, tag="rc", name="rcd")
                nc.vector.reciprocal(rcx[:], src[:, HD:HD + 1])
                nc.scalar.activation(cts[j][:, c, :], src[:, 0:HD], AF.Copy,
                                     scale=rcx[:])
            for j in range(2):
                h = 2 * (OT - 1) + j
                out_dma(out_tiled[:, 4:8, h * HD:(h + 1) * HD], cts[j][:])

def build():
    if "nc" in _CACHE:
        return _CACHE["nc"]
    nc = bacc.Bacc("TRN2", target_bir_lowering=False, debug=False,
                   num_devices=N_CORES)
    with tile.TileContext(nc) as tc:
        _emit(tc)
    nc.compile()
    _CACHE["nc"] = nc
    return nc


def make_in_maps(hidden_state, Wq, bq, Wk, bk, Wv, bv):
    hs = np.ascontiguousarray(np.asarray(hidden_state, dtype=np.float32))
    common = {
        "wq": np.ascontiguousarray(np.asarray(Wq, np.float32)),
        "wk": np.ascontiguousarray(np.asarray(Wk, np.float32)),
        "wv": np.ascontiguousarray(np.asarray(Wv, np.float32)),
        "bq": np.ascontiguousarray(np.asarray(bq, np.float32)),
        "bk": np.ascontiguousarray(np.asarray(bk, np.float32)),
        "bv": np.ascontiguousarray(np.asarray(bv, np.float32)),
    }
    return [{"x": hs[i], **common} for i in range(N_CORES)]


def kernel(hidden_state, attention_mask, Wq, bq, Wk, bk, Wv, bv):
    # attention_mask: per-(batch, query) additive constant -> cancels in
    # softmax (see module docstring); unused.
    nc = build()
    in_maps = make_in_maps(hidden_state, Wq, bq, Wk, bk, Wv, bv)
    res = run_bass_kernel_spmd(nc, in_maps, list(range(N_CORES)))
    return np.stack([res.results[i]["out"] for i in range(N_CORES)], axis=0)
